# revision 10
# baseline (speedup 1.0000x reference)
# DCPLoss Trainium2 Bass kernel (v3).
#
# Computation (see nn_DCPLoss): dark-channel prior loss over [1,8,3,640,640]
# f32 inputs gt/output/output_ema. Sharding: spatial over H across 8 cores
# (80 rows each + reflected halos prepared on host). No collectives: each
# core returns row-partial sums; the host finishes the (tiny) scalar math.
#
# Structure:
#  - Host sends floor(255*x) as EXACT bf16 integers (0..255). The dark-channel
#    min/erosion pipeline runs in bf16 (DVE 2x mode), no on-device quantize.
#  - smap uses the same quantized values: |q_out - q_gt| instead of raw
#    255*|out-gt| (zero-mean quantization error, ~1e-6 final rel err).
#  - Engines: DVE: channel-min, h/v min trees, residual subs, final compares.
#    Pool (GpSimd): smap subs (only add/sub/mult exist there), sq box chain.
#    ACT: abs, PSUM->SBUF copies. PE: frame accumulation of residual/smap
#    maps (identity/selector matmuls into PSUM) + variance-stage banded
#    box-sum matmuls. DMA: loads split per tensor for interleaving; v-tree
#    row shifts.
#  - v-tree batches tapered (2,2,2,1,1) so the tail batch has a short
#    serial DMA-shift chain; double-buffered across batches.
import sys
import numpy as np
import ml_dtypes

sys.path.insert(0, "/opt/trn_rl_repo")

from contextlib import ExitStack

import concourse.bass as bass
import concourse.tile as tile
from concourse import bacc, mybir, bass_utils

A = mybir.AluOpType
F = mybir.ActivationFunctionType
DT = mybir.dt

NCORES = 8
H = W = 640
T = 8           # frames
C = 3           # channels
NT = 3          # tensors: gt, output, ema
ROWS = H // NCORES          # 80 output rows per core
HALO = 10                   # 7 (erosion) + 3 (local var)
RIN = ROWS + 2 * HALO       # 100 input rows per core
WIN = W + 2 * HALO          # 660 input cols
DCW = W + 6                 # 646 dc cols ([-3, 643))
DCR = ROWS + 6              # 86 dc rows
PW2 = NT * DCW              # 1938
BATCHES = [(0, 2), (2, 2), (4, 2), (6, 1), (7, 1)]   # (first frame, nframes)
NBMAX = 2
DLMAX = NBMAX * PW2

_CACHE = {}


def _build_nc():
    nc = bacc.Bacc("TRN2", target_bir_lowering=False, debug=False)
    xin = nc.dram_tensor("xin", [T, NT, C, RIN, WIN], DT.bfloat16, kind="ExternalInput")
    band = nc.dram_tensor("band", [DCR, ROWS], DT.float32, kind="ExternalInput")
    eye = nc.dram_tensor("eye", [DCR, DCR], DT.bfloat16, kind="ExternalInput")
    sel = nc.dram_tensor("sel", [RIN, ROWS], DT.bfloat16, kind="ExternalInput")
    out = nc.dram_tensor("out", [ROWS, 4], DT.float32, kind="ExternalOutput")

    L = NT * WIN            # 1980 cols of chan-min maps (3 tensors)

    with tile.TileContext(nc) as tc, ExitStack() as ctx:
        def pool(nm, bufs=2):
            return ctx.enter_context(tc.tile_pool(name=nm, bufs=bufs))

        ctx2 = ExitStack()

        def fpool(nm, bufs=2):
            return ctx2.enter_context(tc.tile_pool(name=nm, bufs=bufs))

        # persistent tiles
        p_w = pool("w", bufs=1)
        eyet = p_w.tile([DCR, DCR], DT.bfloat16, name="eyet", tag="eyet")
        nc.sync.dma_start(eyet[:], eye.ap())
        bandt = p_w.tile([DCR, ROWS], DT.float32, name="bandt", tag="bandt")
        nc.sync.dma_start(bandt[:], band.ap())
        selt = p_w.tile([RIN, ROWS], DT.bfloat16, name="selt", tag="selt")
        nc.sync.dma_start(selt[:], sel.ap())
        p_fin = pool("fin", bufs=1)

        # PSUM accumulators (live across the frame loop)
        p_ps_sm = ctx.enter_context(tc.tile_pool(name="pssm", bufs=1, space="PSUM"))
        p_ps_racc = ctx2.enter_context(tc.tile_pool(name="psracc", bufs=1, space="PSUM"))
        CH = (512, DCW - 512)   # racc col chunks (PSUM bank <= 512 f32)
        racc_sr = [p_ps_racc.tile([DCR, c], DT.float32, name=f"rsr{i}", tag=f"rsr{i}")
                   for i, c in enumerate(CH)]
        racc_em = [p_ps_racc.tile([DCR, c], DT.float32, name=f"rem{i}", tag=f"rem{i}")
                   for i, c in enumerate(CH)]
        SCH = (512, W - 512)    # smap col chunks
        smap_ps = [p_ps_sm.tile([ROWS, c], DT.float32, name=f"sm{i}", tag=f"sm{i}")
                   for i, c in enumerate(SCH)]

        # per-frame pools
        p_x3 = fpool("x3")
        p_m = fpool("m", bufs=1)
        p_h = fpool("h", bufs=1)
        p_hmv = fpool("hmv", bufs=2)
        p_vs = fpool("vs", bufs=1)
        p_v = fpool("v", bufs=1)
        p_d = fpool("d", bufs=2)
        p_ad = fpool("ad", bufs=2)
        p_sm = fpool("smt", bufs=1)

        dma_engs = [nc.sync, nc.scalar]

        for bi, (t0, nb) in enumerate(BATCHES):
          dl = nb * PW2
          hmv = p_hmv.tile([RIN, DLMAX], DT.bfloat16, name=f"hmv_{bi}", tag="hmv")
          for tf in range(nb):
            t = t0 + tf
            # ---- load: one DMA per tensor (finer DMA interleaving) ----
            x3 = p_x3.tile([RIN, NT * C * WIN], DT.bfloat16, name=f"x3_{t}", tag="x3")
            x3q = x3[:].rearrange("p (n c w) -> p n c w", n=NT, c=C)
            for n in range(NT):
                dma_engs[(t + n) % 2].dma_start(
                    x3q[:, n], xin.ap()[t, n].rearrange("c p w -> p c w"))
            x3v = x3q

            # ---- channel min (DVE bf16 2x; Pool can't do min) ----
            m = p_m.tile([RIN, L], DT.bfloat16, name=f"m_{t}", tag=f"m{t % 2}")
            mv = m[:].rearrange("p (n w) -> p n w", n=NT)
            nc.vector.tensor_tensor(mv, x3v[:, :, 0, :], x3v[:, :, 1, :], A.min)
            nc.vector.tensor_tensor(mv, mv, x3v[:, :, 2, :], A.min)

            # ---- horizontal window-15 min tree (DVE, bf16 2x) ----
            h1 = p_h.tile([RIN, L], DT.bfloat16, name=f"h1_{t}", tag=f"h1{t % 2}")
            nc.vector.tensor_tensor(h1[:, 0:L - 1], m[:, 0:L - 1], m[:, 1:L], A.min)
            h2 = p_h.tile([RIN, L], DT.bfloat16, name=f"h2_{t}", tag=f"h2{t % 2}")
            nc.vector.tensor_tensor(h2[:, 0:L - 3], h1[:, 0:L - 3], h1[:, 2:L - 1], A.min)
            nc.vector.tensor_tensor(h1[:, 0:L - 7], h2[:, 0:L - 7], h2[:, 4:L - 3], A.min)
            h1v = h1[:].rearrange("p (n w) -> p n w", n=NT)
            nc.vector.tensor_tensor(
                hmv[:, tf * PW2:(tf + 1) * PW2].rearrange("p (n w) -> p n w", n=NT),
                h1v[:, :, 0:DCW], h1v[:, :, 7:7 + DCW], A.min)

            # ---- smap path: |q_out - q_gt| summed over (t,c) via PE ----
            d = p_sm.tile([RIN, C * W], DT.bfloat16, name=f"d_{t}", tag=f"d{t % 2}")
            dv = d[:].rearrange("p (c w) -> p c w", c=C)
            nc.gpsimd.tensor_tensor(
                dv,
                x3v[:, 1, :, HALO:HALO + W],
                x3v[:, 0, :, HALO:HALO + W],
                A.subtract)
            ad = p_sm.tile([RIN, C * W], DT.bfloat16, name=f"ad_{t}", tag=f"ad{t % 2}")
            nc.scalar.activation(ad[:], d[:], F.Abs)
            for c in range(C):
                for i, cw in enumerate(SCH):
                    off = c * W + (0 if i == 0 else SCH[0])
                    nc.tensor.matmul(
                        smap_ps[i][:], selt[:], ad[:, off:off + cw],
                        start=(t == 0 and c == 0), stop=(t == T - 1 and c == C - 1))

          # ---- batched vertical window-15 min tree (DMA shifts + DVE) ----
          sfx = f"_{bi}"
          vs1 = p_vs.tile([RIN, DLMAX], DT.bfloat16, name=f"vs1{sfx}", tag=f"vs{bi % 2}")
          dma_engs[bi % 2].dma_start(vs1[0:RIN - 1, 0:dl], hmv[1:RIN, 0:dl])
          v1 = p_v.tile([RIN, DLMAX], DT.bfloat16, name=f"v1{sfx}", tag=f"va{bi % 2}")
          nc.vector.tensor_tensor(v1[0:RIN - 1, 0:dl], hmv[0:RIN - 1, 0:dl],
                                  vs1[0:RIN - 1, 0:dl], A.min)
          vs2 = p_vs.tile([RIN, DLMAX], DT.bfloat16, name=f"vs2{sfx}", tag=f"vs{bi % 2}")
          dma_engs[(bi + 1) % 2].dma_start(vs2[0:RIN - 3, 0:dl], v1[2:RIN - 1, 0:dl])
          v2 = p_v.tile([RIN, DLMAX], DT.bfloat16, name=f"v2{sfx}", tag=f"vb{bi % 2}")
          nc.vector.tensor_tensor(v2[0:RIN - 3, 0:dl], v1[0:RIN - 3, 0:dl],
                                  vs2[0:RIN - 3, 0:dl], A.min)
          vs3 = p_vs.tile([RIN, DLMAX], DT.bfloat16, name=f"vs3{sfx}", tag=f"vs{bi % 2}")
          dma_engs[bi % 2].dma_start(vs3[0:RIN - 7, 0:dl], v2[4:RIN - 3, 0:dl])
          v3 = p_v.tile([RIN, DLMAX], DT.bfloat16, name=f"v3{sfx}", tag=f"va{bi % 2}")
          nc.vector.tensor_tensor(v3[0:RIN - 7, 0:dl], v2[0:RIN - 7, 0:dl],
                                  vs3[0:RIN - 7, 0:dl], A.min)
          vs4 = p_vs.tile([DCR, DLMAX], DT.bfloat16, name=f"vs4{sfx}", tag=f"vs{bi % 2}")
          dma_engs[(bi + 1) % 2].dma_start(vs4[0:DCR, 0:dl], v3[7:RIN - 7, 0:dl])
          v4 = p_v.tile([DCR, DLMAX], DT.bfloat16, name=f"v4{sfx}", tag=f"vb{bi % 2}")
          nc.vector.tensor_tensor(v4[:, 0:dl], v3[0:DCR, 0:dl], vs4[0:DCR, 0:dl], A.min)

          # ---- residuals: |dc_gt - dc_x| (bf16 ints), PE-accumulated ----
          dcv = v4[:, 0:dl].rearrange("p (f n w) -> p f n w", f=nb, n=NT)
          dd = p_d.tile([DCR, 2 * NBMAX * DCW], DT.bfloat16, name=f"dd{sfx}", tag="dd")
          ddv = dd[:, 0:2 * nb * DCW].rearrange("p (k f w) -> p k f w", k=2, f=nb)
          nc.vector.tensor_tensor(ddv[:, 0], dcv[:, :, 0, :], dcv[:, :, 1, :], A.subtract)
          nc.vector.tensor_tensor(ddv[:, 1], dcv[:, :, 0, :], dcv[:, :, 2, :], A.subtract)
          add_ = p_ad.tile([DCR, 2 * NBMAX * DCW], DT.bfloat16, name=f"add{sfx}", tag="add")
          nc.scalar.activation(add_[:, 0:2 * nb * DCW], dd[:, 0:2 * nb * DCW], F.Abs)
          adv = add_[:, 0:2 * nb * DCW].rearrange("p (k f w) -> p k f w", k=2, f=nb)
          for tf in range(nb):
              t = t0 + tf
              for i, cw in enumerate(CH):
                  off = 0 if i == 0 else CH[0]
                  nc.tensor.matmul(
                      racc_sr[i][:], eyet[:], adv[:, 0, tf, off:off + cw],
                      start=(t == 0), stop=(t == T - 1))
                  nc.tensor.matmul(
                      racc_em[i][:], eyet[:], adv[:, 1, tf, off:off + cw],
                      start=(t == 0), stop=(t == T - 1))

        # ================= final stage (once per core) =================
        # racc_sr: [86, 646] f32 ints, rows a-3..b+3, cols -3..643
        rsr = p_fin.tile([DCR, DCW], DT.float32, name="rsr", tag="rsr")
        nc.scalar.activation(rsr[:, 0:CH[0]], racc_sr[0][:], F.Identity)
        nc.scalar.activation(rsr[:, CH[0]:DCW], racc_sr[1][:], F.Identity)
        rem = p_fin.tile([DCR, DCW], DT.float32, name="rem", tag="rem")
        nc.scalar.activation(rem[:, 0:CH[0]], racc_em[0][:], F.Identity)
        nc.scalar.activation(rem[:, CH[0]:DCW], racc_em[1][:], F.Identity)
        sq = p_fin.tile([DCR, DCW], DT.float32, name="sq", tag="sq")
        nc.scalar.activation(sq[:, 0:CH[0]], racc_sr[0][:], F.Square)
        nc.scalar.activation(sq[:, CH[0]:DCW], racc_sr[1][:], F.Square)
        ctx2.close()   # frees per-frame pools + racc PSUM banks

        # center-aligned [80,640] copies (partition offset 3 -> 0 via DMA)
        rsr_c = p_fin.tile([ROWS, W], DT.float32, name="rsr_c", tag="rsr_c")
        nc.scalar.dma_start(rsr_c[:], rsr[3:3 + ROWS, 3:3 + W])
        rem_c = p_fin.tile([ROWS, W], DT.float32, name="rem_c", tag="rem_c")
        nc.sync.dma_start(rem_c[:], rem[3:3 + ROWS, 3:3 + W])

        p_ps = ctx.enter_context(tc.tile_pool(name="psvar", bufs=1, space="PSUM"))

        def box7(src, nm, eng):
            t2 = p_fin.tile([DCR, DCW], DT.float32, name=f"{nm}_t2", tag=f"{nm}_t2")
            eng.tensor_tensor(
                t2[:, 0:DCW - 1], src[:, 0:DCW - 1], src[:, 1:DCW], A.add)
            t4 = p_fin.tile([DCR, DCW], DT.float32, name=f"{nm}_t4", tag=f"{nm}_t4")
            eng.tensor_tensor(
                t4[:, 0:DCW - 3], t2[:, 0:DCW - 3], t2[:, 2:DCW - 1], A.add)
            t6 = p_fin.tile([DCR, W], DT.float32, name=f"{nm}_t6", tag=f"{nm}_t6")
            eng.tensor_tensor(t6[:], t4[:, 0:W], t2[:, 4:4 + W], A.add)
            s7 = p_fin.tile([DCR, W], DT.float32, name=f"{nm}_s7", tag=f"{nm}_s7")
            eng.tensor_tensor(s7[:], t6[:], src[:, 6:6 + W], A.add)
            ps = p_ps.tile([ROWS, W], DT.float32, name=f"{nm}_ps", tag=f"{nm}_ps")
            nc.tensor.matmul(ps[:, 0:512], bandt[:], s7[:, 0:512])
            nc.tensor.matmul(ps[:, 512:W], bandt[:], s7[:, 512:W])
            return ps

        ps1 = box7(rsr, "b1", nc.vector)
        ps2 = box7(sq, "b2", nc.gpsimd)
        s1sq = p_fin.tile([ROWS, W], DT.float32, name="s1sq", tag="s1sq")
        nc.scalar.activation(s1sq[:], ps1[:], F.Square)
        # pw_un = s2b - s1sq/49  (= 49*48*var scaled; sign kept, abs later)
        pw = p_fin.tile([ROWS, W], DT.float32, name="pw", tag="pw")
        nc.vector.scalar_tensor_tensor(
            pw[:], s1sq[:], -1.0 / 49.0, ps2[:], A.mult, A.add)
        wabs = p_fin.tile([ROWS, W], DT.float32, name="wabs", tag="wabs")
        nc.scalar.activation(wabs[:], pw[:], F.Abs)

        mask = p_fin.tile([ROWS, W], DT.float32, name="mask", tag="mask")
        nc.vector.tensor_tensor(mask[:], rsr_c[:], rem_c[:], A.is_ge)
        nc.vector.tensor_tensor(wabs[:], wabs[:], mask[:], A.mult)

        sc = p_fin.tile([ROWS, W], DT.float32, name="sc", tag="sc")
        nc.scalar.activation(sc[:, 0:SCH[0]], smap_ps[0][:], F.Identity)
        nc.scalar.activation(sc[:, SCH[0]:W], smap_ps[1][:], F.Identity)

        # row partials -> one [80,4] tile -> one DMA out
        out4 = p_fin.tile([ROWS, 4], DT.float32, name="out4", tag="out4")
        scr = p_fin.tile([ROWS, W], DT.float32, name="scr", tag="scr")
        nc.vector.tensor_tensor(scr[:], wabs[:], sc[:], A.mult)
        nc.vector.tensor_reduce(out4[:, 2:3], scr[:], mybir.AxisListType.X, A.add)
        nc.vector.tensor_reduce(out4[:, 0:1], rsr_c[:], mybir.AxisListType.X, A.add)
        scr2 = p_fin.tile([ROWS, W], DT.float32, name="scr2", tag="scr2")
        nc.scalar.activation(scr2[:], rsr_c[:], F.Square)
        nc.vector.tensor_reduce(out4[:, 1:2], scr2[:], mybir.AxisListType.X, A.add)
        nc.vector.tensor_copy(out4[:, 3:4], out4[:, 0:1])
        nc.sync.dma_start(out.ap(), out4[:])

    nc.compile()
    return nc


def _band_matrix():
    b = np.zeros((DCR, ROWS), dtype=np.float32)
    for r in range(ROWS):
        b[r:r + 7, r] = 1.0
    return b


def _prep_inputs(gt, output, output_ema):
    full = np.stack([
        np.asarray(gt)[0], np.asarray(output)[0], np.asarray(output_ema)[0]
    ])                                           # [3, 8, 3, 640, 640] f32
    # floor(255*clip(x,0,1)) as exact bf16 integers (u8 cast truncates == floor)
    q = (np.clip(full, 0.0, 1.0) * 255.0).astype(np.uint8)
    q = q.astype(ml_dtypes.bfloat16)
    padded = np.pad(q, ((0, 0), (0, 0), (0, 0), (HALO, HALO), (HALO, HALO)),
                    mode="reflect")              # [3, 8, 3, 660, 660]
    band = _band_matrix()
    eye = np.eye(DCR, dtype=ml_dtypes.bfloat16)
    sel = np.zeros((RIN, ROWS), dtype=ml_dtypes.bfloat16)
    for i in range(ROWS):
        sel[i + HALO, i] = 1
    in_maps = []
    for i in range(NCORES):
        slab = np.ascontiguousarray(
            padded[:, :, :, ROWS * i:ROWS * i + RIN, :].transpose(1, 0, 2, 3, 4))
        in_maps.append({"xin": slab, "band": band, "eye": eye, "sel": sel})
    return in_maps


def _host_finish(outs):
    r1 = sum(float(o["out"][:, 0].astype(np.float64).sum()) for o in outs)
    r2 = sum(float(o["out"][:, 1].astype(np.float64).sum()) for o in outs)
    ws = sum(float(o["out"][:, 2].astype(np.float64).sum()) for o in outs)
    n = float(H * W)
    var_u = (r2 - r1 * r1 / n) / (n - 1.0) / (255.0 ** 2)
    patch_w = var_u ** 0.2
    ntot = float(T * C * H * W)
    loss = patch_w * ws / (48.0 * 255.0 ** 3) / ntot
    return np.float32(loss)


def kernel(**inputs):
    if "nc" not in _CACHE:
        _CACHE["nc"] = _build_nc()
    nc = _CACHE["nc"]
    in_maps = _prep_inputs(inputs["gt"], inputs["output"], inputs["output_ema"])
    res = bass_utils.run_bass_kernel_spmd(nc, in_maps, core_ids=list(range(NCORES)))
    return _host_finish(res.results)
